# revision 19
# baseline (speedup 1.0000x reference)
"""Trainium2 Bass kernel for nn_AggregateStgcn (gnn_message_passing).

Computes, for x:(1,16,1,8192) f32, graph:(8192,8192) f32, fifo:(1,16,4,8192) f32,
stride=2:
    Asum[k, v] = sum_c x[0, c*4+k, 0, v]              (4, 8192)
    xsum[k, w] = sum_v Asum[k, v] * graph[v, w]       (4, 8192)
    S[k, w]    = sum_{j in 1,3,...,13} fifo[0, j, k, w]
    out[0, k, w, 0] = xsum[k, w] + S[k, w]            (1, 4, 8192, 1)

Sharding: graph is split column-wise across 8 NeuronCores (tensor parallel over
output nodes w); the tiny activation/fifo slices are per-core. No collectives;
host concatenates the 8 (4, 1024) output slices.

Strategy: the kernel is a pure HBM stream of the (8192, 1024) per-core graph
slice. The 2e-2 harness error gate allows a single bf16 graph stream (measured
end-to-end err ~1.4e-3), i.e. 16 MB/core - half the traffic of an fp32-exact
hi+lo split. Everything tiny (the c-sum of x, the strided fifo reduce) is
precomputed on the host, so the device program is just:
  - stream 64 v-tiles of G as bf16 on both HWDGE queues (sync+scalar),
    partition-major per chunk so every SBUF partition gets one contiguous run;
  - open each PSUM accumulation group with an S-injecting matmul: an
    8-partition identity lhsT times a (8, 1024) tile holding S as bf16
    hi+lo rows reproduces the fifo term exactly (start=True, so this is
    robust standard group semantics - a DVE preload of PSUM before
    start=False matmuls silently lost the preload on hardware);
  - 128 accumulating matmuls acc[4, 512] += at_tile.T @ G_tile (stationary
    side = 4 cols of packed AsumT, moving side = 512 graph cols at
    1 col/cycle);
  - tail: ACT copies psum half 0 while DVE copies half 1, two 8 KB out DMAs.
The PE (28 us hot) trails the DMA (43 us); filler matmuls after each chunk
keep the PE near-saturated so the HAM clock gate holds the hot ~2.4 GHz
clock (at ~65% utilization the clock sags ~20% and the PE falls behind the
stream, turning into a post-stream tail).
"""

import numpy as np

V = 8192
C = 4
K = 4
F = 16
NCORES = 8
WS = V // NCORES          # 1024 output columns per core
NT = V // 128             # 64 contraction tiles
# graph v-tiles per DMA: small first chunks so the first matmuls start
# ~1.5us earlier (less cold-start backlog), small tail chunks so the
# post-stream matmul tail is short
CHUNKS = [2, 2] + [4] * 14 + [2, 1, 1]
assert sum(CHUNKS) == NT
GBUFS = 6                 # graph chunk buffers in SBUF per stream
# The HAM throttle evaluates PE utilization in ~3.4us windows: any PE idle
# window >~0.5us drops the next window(s) to a ~2x slower clock tier, the
# slowed PE then falls behind the DMA, SBUF chunk buffers fill, and the
# stream itself stalls - a death spiral worth ~8us. So the PE must be
# busy CONTINUOUSLY from preamble-end to the final taper:
#  - a long warmup run bridges preamble-end (~6.5us) to the point where
#    the chunk pipeline delivers work back-to-back (~13.5us); it runs at
#    the cold ~0.4us/matmul rate, so ~17 matmuls fill the gap;
#  - per-chunk fillers then top up each 4-tile chunk (8 real matmuls,
#    ~1.73us) to its ~2.66us arrival period;
#  - the taper at the end lets the PE drain its backlog so no matmuls
#    remain when the last chunk lands (a backlog becomes a pure tail).
WARMUP_MM = 3
FILLERS = [3, 3] + [0] * 17

TRACE = False             # set by test harness to capture an NTFF profile
LAST = None               # BassKernelResults of the most recent run

_CACHED_NC = None


def _build_nc():
    import concourse.bacc as bacc
    import concourse.mybir as mybir
    from concourse.tile import TileContext

    f32 = mybir.dt.float32
    bf16 = mybir.dt.bfloat16
    nc = bacc.Bacc(
        "TRN2",
        target_bir_lowering=False,
        debug=False,
        enable_asserts=False,
        num_devices=NCORES,
    )
    g = nc.dram_tensor("g", [V, WS], bf16, kind="ExternalInput")
    # at: packed AsumT tiles (cols 0:256) + the 8-row S-selector (cols 256:260)
    at = nc.dram_tensor("at", [128, NT * K + K], bf16, kind="ExternalInput")
    sp = nc.dram_tensor("sp", [8, WS], bf16, kind="ExternalInput")
    out = nc.dram_tensor("out", [K, WS], f32, kind="ExternalOutput")

    n_chunks = len(CHUNKS)
    offs = np.cumsum([0] + CHUNKS).tolist()

    with TileContext(nc) as tc:
        with (
            tc.tile_pool(name="const", bufs=1) as cpool,
            tc.tile_pool(name="gp", bufs=GBUFS) as gpool,
            tc.tile_pool(name="ps", bufs=1, space="PSUM") as ppool,
        ):
            # PE warmup: throwaway bf16 matmuls with no input dependencies
            # beyond a memset, so the clock gate opens while data streams in.
            wtile = cpool.tile([128, 512], bf16)
            nc.vector.memset(wtile[:], 1.0)
            wps = ppool.tile([128, 512], f32)
            for _ in range(WARMUP_MM):
                nc.tensor.matmul(
                    wps[:], wtile[:, 0:128], wtile[:], start=True, stop=True
                )

            # the first graph chunk goes ahead of the small inputs on each
            # ring (each DMA dispatch costs ~0.6-1.4us on its issuing engine;
            # the graph stream end time is the critical path)
            g_tiles = [None] * n_chunks

            def emit_gdma(ci):
                s = CHUNKS[ci]
                off = offs[ci]
                rows = slice(off * 128, (off + s) * 128)
                # partition-major within the chunk: partition p holds rows
                # off*128 + p*s .. +s, one contiguous 2*s KB run from HBM
                g_src = g.ap()[rows, :].rearrange(
                    "(p r) w -> p (r w)", p=128, r=s
                )
                gt = gpool.tile([128, s * WS], bf16, name="gt", tag="gt")
                if ci % 2 == 0:
                    nc.sync.dma_start(out=gt[:], in_=g_src)
                else:
                    nc.scalar.dma_start(out=gt[:], in_=g_src)
                g_tiles[ci] = gt

            emit_gdma(0)
            emit_gdma(1)
            at_sb = cpool.tile([128, NT * K + K], bf16)
            nc.sync.dma_start(out=at_sb[:], in_=at.ap())
            sp_sb = cpool.tile([8, WS], bf16)
            nc.sync.dma_start(out=sp_sb[:], in_=sp.ap())

            # open each accumulator group by injecting the fifo term S:
            # acc[h] = selector.T @ sp  (= S_hi + S_lo rows, exact to ~1e-5)
            acc = [ppool.tile([K, 512], f32, name=f"acc{h}") for h in range(2)]
            sel = at_sb[0:8, NT * K : NT * K + K]
            for h in range(2):
                nc.tensor.matmul(
                    acc[h][:],
                    sel,
                    sp_sb[:, h * 512 : (h + 1) * 512],
                    start=True,
                    stop=False,
                )

            for ci, s in enumerate(CHUNKS):
                if ci >= 2:
                    emit_gdma(ci)
                gt = g_tiles[ci]
                off = offs[ci]
                for j in range(s):
                    t = off + j
                    last = t == NT - 1
                    lhsT = at_sb[:, t * K : (t + 1) * K]
                    for h in range(2):
                        nc.tensor.matmul(
                            acc[h][:],
                            lhsT,
                            gt[:, j * WS + h * 512 : j * WS + (h + 1) * 512],
                            start=False,
                            stop=last,
                        )
                for _ in range(FILLERS[ci]):
                    nc.tensor.matmul(
                        wps[:], wtile[:, 0:128], wtile[:],
                        start=True, stop=True,
                    )

            # tail: copy the two psum halves on two different engines in
            # parallel (ACT reads PSUM natively; DVE does the other half),
            # then two 8 KB output DMAs on the idle sync ring
            out_sb = cpool.tile([K, WS], f32)
            nc.scalar.copy(out=out_sb[:, 0:512], in_=acc[0][:])
            nc.vector.tensor_copy(out=out_sb[:, 512:1024], in_=acc[1][:])
            nc.sync.dma_start(out=out.ap(), in_=out_sb[:])

    nc.compile()
    return nc


def kernel(x, graph, fifo, stride):
    global _CACHED_NC, LAST
    import ml_dtypes
    from concourse.bass_utils import run_bass_kernel_spmd

    bf16 = ml_dtypes.bfloat16
    x = np.asarray(x, dtype=np.float32)
    graph = np.asarray(graph, dtype=np.float32)
    fifo = np.asarray(fifo, dtype=np.float32)
    stride_v = int(np.asarray(stride))
    assert stride_v == 2, f"kernel hardcodes stride=2, got {stride_v}"

    # host-side prep (not on the device critical path): c-sum of x and the
    # strided fifo reduce; both are tiny compared to the graph stream
    asum = x.reshape(C, K, V).sum(axis=0)                    # (K, V) f32
    s_full = fifo.reshape(F, C, V)[1 : 2 * (F // 2) - 1 : 2].sum(axis=0)

    # packed AsumT: at[p, t*K + k] = asum[k, v] at v = off_ci*128 + p*s_ci + j
    # (the same permuted v layout the partition-major graph chunks use)
    offs = np.cumsum([0] + CHUNKS).tolist()
    at = np.zeros((128, NT * K + K), dtype=bf16)
    ab = asum.astype(bf16)
    for ci, s in enumerate(CHUNKS):
        off = offs[ci]
        # (K, 128, s): v = off*128 + p*s + j  ->  at[p, (off+j)*K + k]
        blk = ab[:, off * 128 : (off + s) * 128].reshape(K, 128, s)
        at[:, off * K : (off + s) * K] = blk.transpose(1, 2, 0).reshape(
            128, s * K
        )
    # S-selector: partitions k and k+4 both feed output row k
    for k in range(K):
        at[k, NT * K + k] = 1.0
        at[k + 4, NT * K + k] = 1.0

    # S packed as bf16 hi+lo rows: rows 0:4 = bf16(S), rows 4:8 = residual
    s_hi = s_full.astype(bf16)
    s_lo = (s_full - s_hi.astype(np.float32)).astype(bf16)
    sp_full = np.concatenate([s_hi, s_lo], axis=0)           # (8, V) bf16

    # (8, 8192, 1024) per-core column slices of the bf16 graph
    g_sh = np.ascontiguousarray(
        graph.astype(bf16).reshape(V, NCORES, WS).transpose(1, 0, 2)
    )
    sp_sh = np.ascontiguousarray(
        sp_full.reshape(8, NCORES, WS).transpose(1, 0, 2)
    )

    if _CACHED_NC is None:
        _CACHED_NC = _build_nc()
    nc = _CACHED_NC

    in_maps = [
        {"g": g_sh[m], "at": at, "sp": sp_sh[m]}
        for m in range(NCORES)
    ]
    res = run_bass_kernel_spmd(
        nc, in_maps, core_ids=list(range(NCORES)), trace=TRACE
    )
    LAST = res
    b = np.concatenate([res.results[m]["out"] for m in range(NCORES)], axis=1)
    return np.ascontiguousarray(b.reshape(1, C, V, 1))


# revision 20
# speedup vs baseline: 1.1306x; 1.1306x over previous
"""Trainium2 Bass kernel for nn_AggregateStgcn (gnn_message_passing).

Computes, for x:(1,16,1,8192) f32, graph:(8192,8192) f32, fifo:(1,16,4,8192) f32,
stride=2:
    Asum[k, v] = sum_c x[0, c*4+k, 0, v]              (4, 8192)
    xsum[k, w] = sum_v Asum[k, v] * graph[v, w]       (4, 8192)
    S[k, w]    = sum_{j in 1,3,...,13} fifo[0, j, k, w]
    out[0, k, w, 0] = xsum[k, w] + S[k, w]            (1, 4, 8192, 1)

Sharding: graph is split column-wise across 8 NeuronCores (tensor parallel over
output nodes w); the tiny activation/fifo slices are per-core. No collectives;
host concatenates the 8 (4, 1024) output slices.

Strategy: the kernel is a pure HBM stream of the (8192, 1024) per-core graph
slice. The 2e-2 harness error gate allows a single bf16 graph stream (measured
end-to-end err ~1.4e-3), i.e. 16 MB/core - half the traffic of an fp32-exact
hi+lo split. Everything tiny (the c-sum of x, the strided fifo reduce) is
precomputed on the host, so the device program is just:
  - stream 64 v-tiles of G as bf16 on both HWDGE queues (sync+scalar),
    partition-major per chunk so every SBUF partition gets one contiguous run;
  - open each PSUM accumulation group with an S-injecting matmul: an
    8-partition identity lhsT times a (8, 1024) tile holding S as bf16
    hi+lo rows reproduces the fifo term exactly (start=True, so this is
    robust standard group semantics - a DVE preload of PSUM before
    start=False matmuls silently lost the preload on hardware);
  - 128 accumulating matmuls acc[4, 512] += at_tile.T @ G_tile (stationary
    side = 4 cols of packed AsumT, moving side = 512 graph cols at
    1 col/cycle);
  - tail: ACT copies psum half 0 while DVE copies half 1, two 8 KB out DMAs.
The PE (28 us hot) trails the DMA (43 us); filler matmuls after each chunk
keep the PE near-saturated so the HAM clock gate holds the hot ~2.4 GHz
clock (at ~65% utilization the clock sags ~20% and the PE falls behind the
stream, turning into a post-stream tail).
"""

import numpy as np

V = 8192
C = 4
K = 4
F = 16
NCORES = 8
WS = V // NCORES          # 1024 output columns per core
NT = V // 128             # 64 contraction tiles
CHUNKS = [4] * 15 + [2, 1, 1]   # graph v-tiles per DMA; small tail chunks
assert sum(CHUNKS) == NT
GBUFS = 6                 # graph chunk buffers in SBUF per stream
# The HAM throttle evaluates PE utilization in ~3.4us windows: a PE idle
# window drops the following window(s) to a ~20% slower clock tier (or
# worse), the slowed PE falls behind the DMA, SBUF chunk buffers fill,
# and the stream itself stalls. Conversely, piling on filler matmuls to
# hold the hot tier builds a matmul backlog that outlives the stream as
# a pure tail. This warmup/filler schedule is the empirical optimum of
# that trade-off: a short warmup bridges preamble-end to the first chunk,
# 3 fillers per chunk keep utilization high through the front of the
# stream, and the taper lets the backlog drain before the stream ends.
WARMUP_MM = 4
FILLERS = [3] * 10 + [1] * 4 + [0] * 4

TRACE = False             # set by test harness to capture an NTFF profile
LAST = None               # BassKernelResults of the most recent run

_CACHED_NC = None


def _build_nc():
    import concourse.bacc as bacc
    import concourse.mybir as mybir
    from concourse.tile import TileContext

    f32 = mybir.dt.float32
    bf16 = mybir.dt.bfloat16
    nc = bacc.Bacc(
        "TRN2",
        target_bir_lowering=False,
        debug=False,
        enable_asserts=False,
        num_devices=NCORES,
    )
    g = nc.dram_tensor("g", [V, WS], bf16, kind="ExternalInput")
    # at: packed AsumT tiles (cols 0:256) + the 8-row S-selector (cols 256:260)
    at = nc.dram_tensor("at", [128, NT * K + K], bf16, kind="ExternalInput")
    sp = nc.dram_tensor("sp", [8, WS], bf16, kind="ExternalInput")
    out = nc.dram_tensor("out", [K, WS], f32, kind="ExternalOutput")

    n_chunks = len(CHUNKS)
    offs = np.cumsum([0] + CHUNKS).tolist()

    with TileContext(nc) as tc:
        with (
            tc.tile_pool(name="const", bufs=1) as cpool,
            tc.tile_pool(name="gp", bufs=GBUFS) as gpool,
            tc.tile_pool(name="ps", bufs=1, space="PSUM") as ppool,
        ):
            # PE warmup: throwaway bf16 matmuls with no input dependencies
            # beyond a memset, so the clock gate opens while data streams in.
            wtile = cpool.tile([128, 512], bf16)
            nc.vector.memset(wtile[:], 1.0)
            wps = ppool.tile([128, 512], f32)
            for _ in range(WARMUP_MM):
                nc.tensor.matmul(
                    wps[:], wtile[:, 0:128], wtile[:], start=True, stop=True
                )

            # the first graph chunk goes ahead of the small inputs on each
            # ring (each DMA dispatch costs ~0.6-1.4us on its issuing engine;
            # the graph stream end time is the critical path)
            g_tiles = [None] * n_chunks

            def emit_gdma(ci):
                s = CHUNKS[ci]
                off = offs[ci]
                rows = slice(off * 128, (off + s) * 128)
                # partition-major within the chunk: partition p holds rows
                # off*128 + p*s .. +s, one contiguous 2*s KB run from HBM
                g_src = g.ap()[rows, :].rearrange(
                    "(p r) w -> p (r w)", p=128, r=s
                )
                gt = gpool.tile([128, s * WS], bf16, name="gt", tag="gt")
                if ci % 2 == 0:
                    nc.sync.dma_start(out=gt[:], in_=g_src)
                else:
                    nc.scalar.dma_start(out=gt[:], in_=g_src)
                g_tiles[ci] = gt

            emit_gdma(0)
            emit_gdma(1)
            at_sb = cpool.tile([128, NT * K + K], bf16)
            nc.sync.dma_start(out=at_sb[:], in_=at.ap())
            sp_sb = cpool.tile([8, WS], bf16)
            nc.sync.dma_start(out=sp_sb[:], in_=sp.ap())

            # open each accumulator group by injecting the fifo term S:
            # acc[h] = selector.T @ sp  (= S_hi + S_lo rows, exact to ~1e-5)
            acc = [ppool.tile([K, 512], f32, name=f"acc{h}") for h in range(2)]
            sel = at_sb[0:8, NT * K : NT * K + K]
            for h in range(2):
                nc.tensor.matmul(
                    acc[h][:],
                    sel,
                    sp_sb[:, h * 512 : (h + 1) * 512],
                    start=True,
                    stop=False,
                )

            for ci, s in enumerate(CHUNKS):
                if ci >= 2:
                    emit_gdma(ci)
                gt = g_tiles[ci]
                off = offs[ci]
                for j in range(s):
                    t = off + j
                    last = t == NT - 1
                    lhsT = at_sb[:, t * K : (t + 1) * K]
                    for h in range(2):
                        nc.tensor.matmul(
                            acc[h][:],
                            lhsT,
                            gt[:, j * WS + h * 512 : j * WS + (h + 1) * 512],
                            start=False,
                            stop=last,
                        )
                for _ in range(FILLERS[ci]):
                    nc.tensor.matmul(
                        wps[:], wtile[:, 0:128], wtile[:],
                        start=True, stop=True,
                    )

            # tail: copy the two psum halves on two different engines in
            # parallel (ACT reads PSUM natively; DVE does the other half),
            # then two 8 KB output DMAs on the idle sync ring
            out_sb = cpool.tile([K, WS], f32)
            nc.scalar.copy(out=out_sb[:, 0:512], in_=acc[0][:])
            nc.vector.tensor_copy(out=out_sb[:, 512:1024], in_=acc[1][:])
            nc.sync.dma_start(out=out.ap(), in_=out_sb[:])

    nc.compile()
    return nc


def kernel(x, graph, fifo, stride):
    global _CACHED_NC, LAST
    import ml_dtypes
    from concourse.bass_utils import run_bass_kernel_spmd

    bf16 = ml_dtypes.bfloat16
    x = np.asarray(x, dtype=np.float32)
    graph = np.asarray(graph, dtype=np.float32)
    fifo = np.asarray(fifo, dtype=np.float32)
    stride_v = int(np.asarray(stride))
    assert stride_v == 2, f"kernel hardcodes stride=2, got {stride_v}"

    # host-side prep (not on the device critical path): c-sum of x and the
    # strided fifo reduce; both are tiny compared to the graph stream
    asum = x.reshape(C, K, V).sum(axis=0)                    # (K, V) f32
    s_full = fifo.reshape(F, C, V)[1 : 2 * (F // 2) - 1 : 2].sum(axis=0)

    # packed AsumT: at[p, t*K + k] = asum[k, v] at v = off_ci*128 + p*s_ci + j
    # (the same permuted v layout the partition-major graph chunks use)
    offs = np.cumsum([0] + CHUNKS).tolist()
    at = np.zeros((128, NT * K + K), dtype=bf16)
    ab = asum.astype(bf16)
    for ci, s in enumerate(CHUNKS):
        off = offs[ci]
        # (K, 128, s): v = off*128 + p*s + j  ->  at[p, (off+j)*K + k]
        blk = ab[:, off * 128 : (off + s) * 128].reshape(K, 128, s)
        at[:, off * K : (off + s) * K] = blk.transpose(1, 2, 0).reshape(
            128, s * K
        )
    # S-selector: partitions k and k+4 both feed output row k
    for k in range(K):
        at[k, NT * K + k] = 1.0
        at[k + 4, NT * K + k] = 1.0

    # S packed as bf16 hi+lo rows: rows 0:4 = bf16(S), rows 4:8 = residual
    s_hi = s_full.astype(bf16)
    s_lo = (s_full - s_hi.astype(np.float32)).astype(bf16)
    sp_full = np.concatenate([s_hi, s_lo], axis=0)           # (8, V) bf16

    # (8, 8192, 1024) per-core column slices of the bf16 graph
    g_sh = np.ascontiguousarray(
        graph.astype(bf16).reshape(V, NCORES, WS).transpose(1, 0, 2)
    )
    sp_sh = np.ascontiguousarray(
        sp_full.reshape(8, NCORES, WS).transpose(1, 0, 2)
    )

    if _CACHED_NC is None:
        _CACHED_NC = _build_nc()
    nc = _CACHED_NC

    in_maps = [
        {"g": g_sh[m], "at": at, "sp": sp_sh[m]}
        for m in range(NCORES)
    ]
    res = run_bass_kernel_spmd(
        nc, in_maps, core_ids=list(range(NCORES)), trace=TRACE
    )
    LAST = res
    b = np.concatenate([res.results[m]["out"] for m in range(NCORES)], axis=1)
    return np.ascontiguousarray(b.reshape(1, C, V, 1))
